# revision 3
# baseline (speedup 1.0000x reference)
"""GQA causal attention (batch 2, seq 2048, hidden 2048, 16 Q heads / 4 KV heads,
head_dim 128) on 8 trn2 NeuronCores.

Sharding: data-parallel over batch (2) x tensor-parallel over KV-head groups (4).
Core c = b*4 + g handles batch b, Q heads [4g, 4g+4), KV head g, and produces a
partial output x_attn @ wo[512g:512g+512, :]; the host sums the 4 partials per
batch (the unshard step for row-sharded o_proj input).

On-chip layout strategy: everything lives transposed so no large transposes are
needed. Host passes x^T [hidden, seq]. Projections compute qT/kT/vT [d, seq]
(weights stationary). Scores are computed transposed [k, q] so that:
  - exp runs on ScalarE psum->sbuf,
  - softmax denominators are partition-reductions done with a ones-column matmul,
  - PV^T = v[k,d]^T-contraction consumes probsT [k, q] directly,
  - PV^T output [d, q] is exactly the lhsT layout o_proj needs.
Normalization (1/rowsum) is broadcast across partitions with a rank-1 matmul.
Causality: score blocks strictly above the diagonal are skipped entirely; the
4 distinct 128x512 diagonal-crossing patterns are 0/1 bf16 masks multiplied in
after exp (exp(s)*m == exp(s + log m) matches the reference's additive -1e9).
"""

import sys

sys.path.insert(0, "/opt/trn_rl_repo")

from contextlib import ExitStack

import ml_dtypes
import numpy as np

import concourse.bass as bass  # noqa: F401  (import keeps bass registered)
import concourse.mybir as mybir
import concourse.tile as tile
from concourse import bacc
from concourse.bass_utils import run_bass_kernel_spmd
from concourse.masks import make_identity

BF16 = mybir.dt.bfloat16
F32 = mybir.dt.float32
NPBF16 = ml_dtypes.bfloat16

B = 2
S = 2048
H = 2048
D = 128
N_HEADS = 16
N_KV = 4
QC = 512  # q columns per core (4 heads x 128)
NHC = H // 128  # 16 hidden chunks
NRB = S // 512  # 4 row blocks of 512
NQC = QC // 128  # 4 head chunks per core
NKC = S // 128  # 16 key chunks of 128
SCALE = 1.0 / float(np.sqrt(D))

_NC = None


def _build_nc():
    nc = bacc.Bacc("TRN2", target_bir_lowering=False, debug=False, num_devices=8)

    xT = nc.dram_tensor("xT", [H, S], BF16, kind="ExternalInput")
    wq = nc.dram_tensor("wq", [H, QC], BF16, kind="ExternalInput")
    wk = nc.dram_tensor("wk", [H, D], BF16, kind="ExternalInput")
    wv = nc.dram_tensor("wv", [H, D], BF16, kind="ExternalInput")
    wo = nc.dram_tensor("wo", [QC, H], BF16, kind="ExternalInput")
    bq = nc.dram_tensor("bq", [QC, 1], F32, kind="ExternalInput")
    bk = nc.dram_tensor("bk", [D, 1], F32, kind="ExternalInput")
    bv = nc.dram_tensor("bv", [D, 1], F32, kind="ExternalInput")
    masks = nc.dram_tensor("masks", [4, 128, 512], BF16, kind="ExternalInput")
    out = nc.dram_tensor("out", [S, H], F32, kind="ExternalOutput")

    with tile.TileContext(nc) as tc, ExitStack() as ctx:
        persist = ctx.enter_context(tc.tile_pool(name="persist", bufs=1))

        qT = [persist.tile([128, S], BF16, tag=f"qT{i}", name=f"qT{i}") for i in range(NQC)]
        kT = persist.tile([128, S], BF16, tag="kT", name="kT")
        v_sb = persist.tile([128, NKC, D], BF16, tag="v", name="v_sb")
        AT = [persist.tile([128, S], BF16, tag=f"AT{i}", name=f"AT{i}") for i in range(NQC)]
        mask_sb = persist.tile([128, 4, 512], BF16, tag="masks", name="mask_sb")
        bq_sb = persist.tile([128, NQC], F32, tag="bq", name="bq_sb")
        bk_sb = persist.tile([128, 1], F32, tag="bk", name="bk_sb")
        bv_sb = persist.tile([128, 1], F32, tag="bv", name="bv_sb")
        ones_col = persist.tile([128, 1], BF16, tag="onesc", name="ones_col")
        ones_row = persist.tile([1, 128], BF16, tag="onesr", name="ones_row")
        ident = persist.tile([128, 128], BF16, tag="ident", name="ident")

        nc.vector.memset(ones_col, 1.0)
        nc.vector.memset(ones_row, 1.0)
        make_identity(nc, ident)
        for vv in range(4):
            nc.sync.dma_start(out=mask_sb[:, vv, :], in_=masks[vv])
        for qc in range(NQC):
            nc.sync.dma_start(
                out=bq_sb[:, qc : qc + 1], in_=bq[qc * 128 : (qc + 1) * 128, :]
            )
        nc.sync.dma_start(out=bk_sb, in_=bk[:, :])
        nc.sync.dma_start(out=bv_sb, in_=bv[:, :])

        # ---------------- Phase 1: projections ----------------
        with tc.tile_pool(name="p1", bufs=1) as p1, tc.tile_pool(
            name="psum1", bufs=8, space="PSUM"
        ) as pp1:
            xt_sb = [p1.tile([128, S], BF16, tag=f"x{hc}", name=f"x{hc}") for hc in range(NHC)]
            wq_sb = [p1.tile([128, QC], BF16, tag=f"wq{hc}", name=f"wq{hc}") for hc in range(NHC)]
            wk_sb = [p1.tile([128, D], BF16, tag=f"wk{hc}", name=f"wk{hc}") for hc in range(NHC)]
            wv_sb = [p1.tile([128, D], BF16, tag=f"wv{hc}", name=f"wv{hc}") for hc in range(NHC)]
            vT = p1.tile([128, S], BF16, tag="vT", name="vT")

            # DMAs in consumption order so compute can start early.
            for hc in range(NHC):
                nc.sync.dma_start(out=wk_sb[hc], in_=wk[hc * 128 : (hc + 1) * 128, :])
                nc.sync.dma_start(out=wv_sb[hc], in_=wv[hc * 128 : (hc + 1) * 128, :])
                nc.sync.dma_start(out=xt_sb[hc], in_=xT[hc * 128 : (hc + 1) * 128, :])
                nc.sync.dma_start(out=wq_sb[hc], in_=wq[hc * 128 : (hc + 1) * 128, :])

            # K/V wave: 8 psum groups, hidden-chunk-major for xT streaming.
            kps = [pp1.tile([128, 512], F32, tag="pp", name="pp") for _ in range(NRB)]
            vps = [pp1.tile([128, 512], F32, tag="pp", name="pp") for _ in range(NRB)]
            for hc in range(NHC):
                st = hc == 0
                sp = hc == NHC - 1
                for rb in range(NRB):
                    rhs = xt_sb[hc][:, rb * 512 : (rb + 1) * 512]
                    nc.tensor.matmul(kps[rb], wk_sb[hc], rhs, start=st, stop=sp)
                    nc.tensor.matmul(vps[rb], wv_sb[hc], rhs, start=st, stop=sp)
            for rb in range(NRB):
                sl = slice(rb * 512, (rb + 1) * 512)
                nc.vector.tensor_scalar_add(kT[:, sl], kps[rb], bk_sb)
                nc.vector.tensor_scalar_add(vT[:, sl], vps[rb], bv_sb)

            # v natural [k, d] via PE transposes of vT 128x128 blocks.
            for kc in range(NKC):
                tps = pp1.tile([128, 128], BF16, tag="pp", name="tp", padded_shape=[128, 1024])
                nc.tensor.transpose(tps, vT[:, kc * 128 : (kc + 1) * 128], ident)
                nc.vector.tensor_copy(v_sb[:, kc, :], tps)

            # Q waves: per row-block, 4 psum groups (one per head chunk).
            for rb in range(NRB):
                qps = [pp1.tile([128, 512], F32, tag="pp", name="qp") for _ in range(NQC)]
                for hc in range(NHC):
                    st = hc == 0
                    sp = hc == NHC - 1
                    rhs = xt_sb[hc][:, rb * 512 : (rb + 1) * 512]
                    for qc in range(NQC):
                        lhsT = wq_sb[hc][:, qc * 128 : (qc + 1) * 128]
                        nc.tensor.matmul(qps[qc], lhsT, rhs, start=st, stop=sp)
                for qc in range(NQC):
                    nc.vector.tensor_scalar_add(
                        qT[qc][:, rb * 512 : (rb + 1) * 512],
                        qps[qc],
                        bq_sb[:, qc : qc + 1],
                    )

        # ---------------- Phase 2+3: attention + o_proj ----------------
        with tc.tile_pool(name="p2", bufs=1) as p2, tc.tile_pool(
            name="p2b", bufs=2
        ) as p2b, tc.tile_pool(name="ps_s", bufs=2, space="PSUM") as ps_s, tc.tile_pool(
            name="ps_pv", bufs=2, space="PSUM"
        ) as ps_pv, tc.tile_pool(
            name="ps_den", bufs=1, space="PSUM"
        ) as ps_den, tc.tile_pool(
            name="ps_b", bufs=1, space="PSUM"
        ) as ps_b, tc.tile_pool(
            name="ps_o", bufs=2, space="PSUM"
        ) as ps_o:
            wo_sb = [p2.tile([128, H], BF16, tag=f"wo{qc}", name=f"wo{qc}") for qc in range(NQC)]
            for qc in range(NQC):
                nc.sync.dma_start(out=wo_sb[qc], in_=wo[qc * 128 : (qc + 1) * 128, :])

            for qg in range(NRB):
                nkc = 4 * (qg + 1)
                qsl = slice(qg * 512, (qg + 1) * 512)
                for h in range(NQC):
                    pT = p2b.tile([128, NKC, 512], BF16, tag="pT", name="pT")
                    for kc in range(nkc):
                        sps = ps_s.tile([128, 512], F32, tag="s", name="sps")
                        nc.tensor.matmul(
                            sps,
                            kT[:, kc * 128 : (kc + 1) * 128],
                            qT[h][:, qsl],
                            start=True,
                            stop=True,
                        )
                        nc.scalar.activation(
                            pT[:, kc, :],
                            sps,
                            mybir.ActivationFunctionType.Exp,
                            scale=SCALE,
                        )
                        if kc >= 4 * qg:
                            nc.vector.tensor_mul(
                                pT[:, kc, :], pT[:, kc, :], mask_sb[:, kc - 4 * qg, :]
                            )
                    pvps = ps_pv.tile([128, 512], F32, tag="pv", name="pvps")
                    denps = ps_den.tile([1, 512], F32, tag="den", name="denps")
                    for kc in range(nkc):
                        st = kc == 0
                        sp = kc == nkc - 1
                        nc.tensor.matmul(
                            pvps, v_sb[:, kc, :], pT[:, kc, :], start=st, stop=sp
                        )
                        nc.tensor.matmul(
                            denps, ones_col, pT[:, kc, :], start=st, stop=sp
                        )
                    recip = p2b.tile([1, 512], F32, tag="recip", name="recip")
                    nc.vector.reciprocal(recip, denps)
                    recip_bf = p2b.tile([1, 512], BF16, tag="recipb", name="recip_bf")
                    nc.vector.tensor_copy(recip_bf, recip)
                    bps = ps_b.tile([128, 512], F32, tag="b", name="bps")
                    nc.tensor.matmul(bps, ones_row, recip_bf, start=True, stop=True)
                    bcast_sb = p2b.tile([128, 512], BF16, tag="bc", name="bcast_sb")
                    nc.scalar.copy(bcast_sb, bps)
                    nc.vector.tensor_mul(AT[h][:, qsl], pvps, bcast_sb)

                # o_proj rows for this q group (partial over head group).
                for rc in range(4 * qg, 4 * qg + 4):
                    rsl = slice(rc * 128, (rc + 1) * 128)
                    for oc in range(NRB):
                        ops = ps_o.tile([128, 512], F32, tag="o", name="ops")
                        osl = slice(oc * 512, (oc + 1) * 512)
                        for qc in range(NQC):
                            nc.tensor.matmul(
                                ops,
                                AT[qc][:, rsl],
                                wo_sb[qc][:, osl],
                                start=(qc == 0),
                                stop=(qc == NQC - 1),
                            )
                        ostage = p2b.tile([128, 512], F32, tag="ost", name="ostage")
                        nc.scalar.copy(ostage, ops)
                        nc.sync.dma_start(out=out[rsl, osl], in_=ostage)

    nc.compile()
    return nc


def _get_nc():
    global _NC
    if _NC is None:
        _NC = _build_nc()
    return _NC


def _make_in_maps(x, wq, bq, wk, bk, wv, bv, wo):
    qli = np.arange(512, dtype=np.int32)[None, :]
    kli = np.arange(128, dtype=np.int32)[:, None]
    mask_np = np.stack(
        [(qli >= kli + 128 * vv) for vv in range(4)]
    ).astype(NPBF16)

    in_maps = []
    for b in range(B):
        xTb = np.ascontiguousarray(np.asarray(x[b], dtype=np.float32).T).astype(NPBF16)
        for g in range(N_KV):
            in_maps.append(
                {
                    "xT": xTb,
                    "wq": np.ascontiguousarray(wq[:, g * 512 : (g + 1) * 512]).astype(
                        NPBF16
                    ),
                    "wk": np.ascontiguousarray(wk[:, g * 128 : (g + 1) * 128]).astype(
                        NPBF16
                    ),
                    "wv": np.ascontiguousarray(wv[:, g * 128 : (g + 1) * 128]).astype(
                        NPBF16
                    ),
                    "wo": np.ascontiguousarray(wo[g * 512 : (g + 1) * 512, :]).astype(
                        NPBF16
                    ),
                    "bq": np.asarray(bq[g * 512 : (g + 1) * 512], dtype=np.float32)
                    .reshape(512, 1)
                    .copy(),
                    "bk": np.asarray(bk[g * 128 : (g + 1) * 128], dtype=np.float32)
                    .reshape(128, 1)
                    .copy(),
                    "bv": np.asarray(bv[g * 128 : (g + 1) * 128], dtype=np.float32)
                    .reshape(128, 1)
                    .copy(),
                    "masks": mask_np,
                }
            )
    return in_maps


def run_device(x, wq, bq, wk, bk, wv, bv, wo, trace=False):
    """Run the SPMD kernel; returns (full_output, BassKernelResults)."""
    nc = _get_nc()
    in_maps = _make_in_maps(x, wq, bq, wk, bk, wv, bv, wo)
    res = run_bass_kernel_spmd(nc, in_maps, core_ids=list(range(8)), trace=trace)
    out = np.zeros((B, S, H), dtype=np.float32)
    for b in range(B):
        for g in range(N_KV):
            out[b] += res.results[b * N_KV + g]["out"]
    return out, res


def kernel(x, attention_mask, position_ids, wq, bq, wk, bk, wv, bv, wo):
    del attention_mask, position_ids  # causal mask is built on-chip
    x = np.asarray(x, dtype=np.float32)
    out, _ = run_device(
        x,
        np.asarray(wq, np.float32),
        np.asarray(bq, np.float32),
        np.asarray(wk, np.float32),
        np.asarray(bk, np.float32),
        np.asarray(wv, np.float32),
        np.asarray(bv, np.float32),
        np.asarray(wo, np.float32),
    )
    return out


# revision 44
# speedup vs baseline: 1.5751x; 1.5751x over previous
"""GQA causal attention (batch 2, seq 2048, hidden 2048, 16 Q heads / 4 KV heads,
head_dim 128) on 8 trn2 NeuronCores.

Sharding: data-parallel over batch (2) x tensor-parallel over KV-head groups (4).
Core c = b*4 + g handles batch b, Q heads [4g, 4g+4), KV head g, and produces a
partial output x_attn @ wo[512g:512g+512, :]; the host sums the 4 partials per
batch (the unshard step for row-sharded o_proj input).

On-chip layout strategy: everything lives transposed so no large transposes are
needed. Host passes x^T [hidden, seq]. Projections compute qT/kT/vT [d, seq]
(weights stationary). Scores are computed transposed [k, q] so that:
  - exp runs on ScalarE psum->sbuf,
  - softmax denominators are partition-reductions done with a ones-column matmul,
  - PV^T = v[k,d]^T-contraction consumes probsT [k, q] directly,
  - PV^T output [d, q] is exactly the lhsT layout o_proj needs.
Normalization (1/rowsum) is broadcast across partitions with a rank-1 matmul.
Causality: score blocks strictly above the diagonal are skipped entirely; the
4 distinct 128x512 diagonal-crossing patterns are 0/1 bf16 masks multiplied in
after exp (exp(s)*m == exp(s + log m) matches the reference's additive -1e9).
"""

import sys

sys.path.insert(0, "/opt/trn_rl_repo")

from contextlib import ExitStack

import ml_dtypes
import numpy as np

import concourse.bass as bass  # noqa: F401  (import keeps bass registered)
import concourse.mybir as mybir
import concourse.tile as tile
from concourse import bacc
from concourse.bass_utils import run_bass_kernel_spmd
from concourse.masks import make_identity

BF16 = mybir.dt.bfloat16
F32 = mybir.dt.float32
NPBF16 = ml_dtypes.bfloat16

B = 2
S = 2048
H = 2048
D = 128
N_HEADS = 16
N_KV = 4
QC = 512  # q columns per core (4 heads x 128)
NHC = H // 128  # 16 hidden chunks
NRB = S // 512  # 4 row blocks of 512
NQC = QC // 128  # 4 head chunks per core
NKC = S // 128  # 16 key chunks of 128
SCALE = 1.0 / float(np.sqrt(D))

_NC = None


def _build_nc():
    nc = bacc.Bacc("TRN2", target_bir_lowering=False, debug=False, num_devices=8)

    xT = nc.dram_tensor("xT", [H, S], BF16, kind="ExternalInput")
    wq = nc.dram_tensor("wq", [H, QC], BF16, kind="ExternalInput")
    wk = nc.dram_tensor("wk", [H, D], BF16, kind="ExternalInput")
    wv = nc.dram_tensor("wv", [H, D], BF16, kind="ExternalInput")
    wo = nc.dram_tensor("wo", [QC, H], BF16, kind="ExternalInput")
    bq = nc.dram_tensor("bq", [QC, 1], F32, kind="ExternalInput")
    bk = nc.dram_tensor("bk", [D, 1], F32, kind="ExternalInput")
    bv = nc.dram_tensor("bv", [D, 1], F32, kind="ExternalInput")
    masks = nc.dram_tensor("masks", [4, 128, 512], BF16, kind="ExternalInput")
    out = nc.dram_tensor("out", [S, H], BF16, kind="ExternalOutput")

    with tile.TileContext(nc) as tc, ExitStack() as ctx:
        persist = ctx.enter_context(tc.tile_pool(name="persist", bufs=1))

        qT = [persist.tile([128, S], BF16, tag=f"qT{i}", name=f"qT{i}") for i in range(NQC)]
        kT = persist.tile([128, S], BF16, tag="kT", name="kT")
        v_sb = persist.tile([128, NKC, D], BF16, tag="v", name="v_sb")
        AT = [persist.tile([128, S], BF16, tag=f"AT{i}", name=f"AT{i}") for i in range(NQC)]
        mask_sb = persist.tile([128, 4, 512], BF16, tag="masks", name="mask_sb")
        bq_sb = persist.tile([128, NQC], F32, tag="bq", name="bq_sb")
        bk_sb = persist.tile([128, 1], F32, tag="bk", name="bk_sb")
        bv_sb = persist.tile([128, 1], F32, tag="bv", name="bv_sb")
        ones_mat = persist.tile([128, 128], BF16, tag="onesm", name="ones_mat")
        ident = persist.tile([128, 128], BF16, tag="ident", name="ident")

        nc.vector.memset(ones_mat, 1.0)
        make_identity(nc, ident)
        for vv in range(4):
            nc.gpsimd.dma_start(out=mask_sb[:, vv, :], in_=masks[vv])
        for qc in range(NQC):
            nc.gpsimd.dma_start(
                out=bq_sb[:, qc : qc + 1], in_=bq[qc * 128 : (qc + 1) * 128, :]
            )
        nc.gpsimd.dma_start(out=bk_sb, in_=bk[:, :])
        nc.gpsimd.dma_start(out=bv_sb, in_=bv[:, :])

        # ---------------- Phase 1: projections ----------------
        with tc.tile_pool(name="p1", bufs=1) as p1, tc.tile_pool(
            name="psum1", bufs=8, space="PSUM"
        ) as pp1:
            xt_sb = [p1.tile([128, S], BF16, tag=f"x{hc}", name=f"x{hc}") for hc in range(NHC)]
            wq_sb = [p1.tile([128, QC], BF16, tag=f"wq{hc}", name=f"wq{hc}") for hc in range(NHC)]
            wk_sb = [p1.tile([128, D], BF16, tag=f"wk{hc}", name=f"wk{hc}") for hc in range(NHC)]
            wv_sb = [p1.tile([128, D], BF16, tag=f"wv{hc}", name=f"wv{hc}") for hc in range(NHC)]
            vT = p1.tile([128, S], BF16, tag="vT", name="vT")

            # DMAs in consumption order, spread across engine queues for
            # bandwidth (each engine issues to its own HWDGE queue).
            for hc in range(NHC):
                if hc == 0:
                    nc.sync.dma_start(out=xt_sb[0][:, :512], in_=xT[:128, :512])
                    nc.sync.dma_start(
                        out=xt_sb[0][:, 512:1024], in_=xT[:128, 512:1024]
                    )
                else:
                    nc.sync.dma_start(
                        out=xt_sb[hc][:, :1024],
                        in_=xT[hc * 128 : (hc + 1) * 128, :1024],
                    )
                nc.scalar.dma_start(out=wq_sb[hc], in_=wq[hc * 128 : (hc + 1) * 128, :])
                nc.gpsimd.dma_start(out=wk_sb[hc], in_=wk[hc * 128 : (hc + 1) * 128, :])
                nc.gpsimd.dma_start(out=wv_sb[hc], in_=wv[hc * 128 : (hc + 1) * 128, :])
            for hc in range(NHC):
                nc.sync.dma_start(
                    out=xt_sb[hc][:, 1024:], in_=xT[hc * 128 : (hc + 1) * 128, 1024:]
                )

            # Q waves first (dense PE work absorbs the xT DMA stream):
            # 2 waves of 8 psum groups (4 head chunks x 2 row blocks).
            for wave in range(2):
                rbs = [2 * wave, 2 * wave + 1]
                qps = {
                    (qc, rb): pp1.tile([128, 512], F32, tag="pp", name="qp")
                    for qc in range(NQC)
                    for rb in rbs
                }
                for hc in range(NHC):
                    st = hc == 0
                    sp = hc == NHC - 1
                    for qc in range(NQC):
                        lhsT = wq_sb[hc][:, qc * 128 : (qc + 1) * 128]
                        for rb in rbs:
                            rhs = xt_sb[hc][:, rb * 512 : (rb + 1) * 512]
                            nc.tensor.matmul(qps[qc, rb], lhsT, rhs, start=st, stop=sp)
                for qc in range(NQC):
                    for rb in rbs:
                        nc.vector.tensor_scalar_add(
                            qT[qc][:, rb * 512 : (rb + 1) * 512],
                            qps[qc, rb],
                            bq_sb[:, qc : qc + 1],
                        )

            # K/V wave: 8 psum groups, hidden-chunk-major.
            kps = [pp1.tile([128, 512], F32, tag="pp", name="pp") for _ in range(NRB)]
            vps = [pp1.tile([128, 512], F32, tag="pp", name="pp") for _ in range(NRB)]
            for hc in range(NHC):
                st = hc == 0
                sp = hc == NHC - 1
                for rb in range(NRB):
                    rhs = xt_sb[hc][:, rb * 512 : (rb + 1) * 512]
                    nc.tensor.matmul(kps[rb], wk_sb[hc], rhs, start=st, stop=sp)
                for rb in range(NRB):
                    rhs = xt_sb[hc][:, rb * 512 : (rb + 1) * 512]
                    nc.tensor.matmul(vps[rb], wv_sb[hc], rhs, start=st, stop=sp)
            for rb in range(NRB):
                sl = slice(rb * 512, (rb + 1) * 512)
                nc.vector.tensor_scalar_add(kT[:, sl], kps[rb], bk_sb)
                nc.vector.tensor_scalar_add(vT[:, sl], vps[rb], bv_sb)

            # v natural [k, d] via PE transposes of vT 128x128 blocks.
            for kc in range(NKC):
                tps = pp1.tile([128, 128], BF16, tag="pp", name="tp", padded_shape=[128, 1024])
                nc.tensor.transpose(tps, vT[:, kc * 128 : (kc + 1) * 128], ident)
                nc.vector.tensor_copy(v_sb[:, kc, :], tps)

        # ---------------- Phase 2: attention ----------------
        with tc.tile_pool(name="p2", bufs=1) as p2, tc.tile_pool(
            name="p2b", bufs=2
        ) as p2b:
            wo_sb = [p2.tile([128, H], BF16, tag=f"wo{qc}", name=f"wo{qc}") for qc in range(NQC)]
            for qc in range(NQC):
                nc.gpsimd.dma_start(out=wo_sb[qc], in_=wo[qc * 128 : (qc + 1) * 128, :])

            att_pools = ExitStack()
            ps_s = att_pools.enter_context(tc.tile_pool(name="ps_s", bufs=2, space="PSUM"))
            ps_pv = att_pools.enter_context(tc.tile_pool(name="ps_pv", bufs=2, space="PSUM"))
            ps_den = att_pools.enter_context(tc.tile_pool(name="ps_den", bufs=1, space="PSUM"))
            ps_oi = att_pools.enter_context(tc.tile_pool(name="ps_oi", bufs=1, space="PSUM"))
            def oproj_inject_ops(rc):
                """Single-bank o_proj ops for one row chunk: 4 groups of
                (4 matmuls + DVE cast + DMA), injected into the next q
                group's ACT-paced attention steps."""
                ops = []
                rsl = slice(rc * 128, (rc + 1) * 128)
                state = {}
                for oc in range(NRB):
                    osl = slice(oc * 512, (oc + 1) * 512)
                    for qc in range(NQC):
                        def mk(qc=qc, oc=oc, osl=osl, rsl=rsl):
                            if qc == 0:
                                state[oc] = ps_oi.tile(
                                    [128, 512], F32, tag="oi", name="oips"
                                )
                            nc.tensor.matmul(
                                state[oc],
                                AT[qc][:, rsl],
                                wo_sb[qc][:, osl],
                                start=(qc == 0),
                                stop=(qc == NQC - 1),
                            )
                        ops.append(mk)
                    def mkcp(oc=oc, osl=osl, rsl=rsl):
                        ostage = p2b.tile(
                            [128, 512], BF16, tag="osti", name="ostagei", bufs=2
                        )
                        nc.vector.tensor_copy(ostage, state.pop(oc))
                        nc.gpsimd.dma_start(out=out[rsl, osl], in_=ostage)
                    ops.append(mkcp)
                return ops

            INJECated = (0, 4, 8)
            pending = []
            for qg in range(NRB):
                nkc = 4 * (qg + 1)
                qsl = slice(qg * 512, (qg + 1) * 512)
                for h in range(NQC):
                    pT = p2b.tile([128, NKC, 512], BF16, tag="pT", name="pT")
                    pvps = ps_pv.tile([128, 512], F32, tag="pv", name="pvps")
                    denps = ps_den.tile([128, 512], F32, tag="den", name="denps")

                    # Column offset of the causal boundary inside block kc:
                    # columns [0:off) are fully masked -> skip them everywhere.
                    def off(kc, qg=qg):
                        return 128 * (kc - 4 * qg) if kc >= 4 * qg else 0

                    # Dead columns [0:o) of diagonal chunks stay zero so the
                    # quad-sums below can read full rows.
                    for kc in range(4 * qg + 1, nkc):
                        nc.vector.memset(pT[:, kc, : off(kc)], 0.0)

                    # Software pipeline over "units": off-diagonal chunks in
                    # pairs sharing one [128,1024] psum tile and ONE exp
                    # (halves ScalarE's per-instruction overhead); diagonal
                    # chunks as column-restricted singles. pv MMs lag 2 units;
                    # every 4th chunk closes a quad-sum feeding one
                    # denominator matmul.
                    units = [(2 * u, 2 * u + 1) for u in range(2 * qg)]
                    units += [(kc,) for kc in range(4 * qg, nkc)]
                    nu = len(units)
                    LAGU = 2
                    for ui in range(nu + LAGU):
                        if ui < nu:
                            unit = units[ui]
                            sps = ps_s.tile([128, 1024], F32, tag="s", name="sps")
                            if len(unit) == 2:
                                a, b = unit
                                nc.tensor.matmul(
                                    sps[:, :512],
                                    kT[:, a * 128 : (a + 1) * 128],
                                    qT[h][:, qsl],
                                    start=True,
                                    stop=True,
                                )
                                nc.tensor.matmul(
                                    sps[:, 512:],
                                    kT[:, b * 128 : (b + 1) * 128],
                                    qT[h][:, qsl],
                                    start=True,
                                    stop=True,
                                )
                                nc.scalar.activation(
                                    pT[:, a : a + 2, :],
                                    sps[:, :],
                                    mybir.ActivationFunctionType.Exp,
                                    scale=SCALE,
                                )
                            else:
                                kc = unit[0]
                                o = off(kc)
                                nc.tensor.matmul(
                                    sps[:, o:512],
                                    kT[:, kc * 128 : (kc + 1) * 128],
                                    qT[h][:, qg * 512 + o : (qg + 1) * 512],
                                    start=True,
                                    stop=True,
                                )
                                nc.scalar.activation(
                                    pT[:, kc, o:],
                                    sps[:, o:512],
                                    mybir.ActivationFunctionType.Exp,
                                    scale=SCALE,
                                )
                                # only [o, o+128) can contain masked elements
                                nc.vector.tensor_mul(
                                    pT[:, kc, o : o + 128],
                                    pT[:, kc, o : o + 128],
                                    mask_sb[:, kc - 4 * qg, o : o + 128],
                                )
                        if pending:
                            pending.pop(0)()
                        uj = ui - LAGU
                        if 0 <= uj < nu:
                            for j in units[uj]:
                                oj = off(j)
                                nc.tensor.matmul(
                                    pvps[:, oj:],
                                    v_sb[:, j, :],
                                    pT[:, j, oj:],
                                    start=(j == 0),
                                    stop=(j == nkc - 1),
                                )
                                if j % 2 == 1:
                                    m = j // 2
                                    tq = p2b.tile([128, 512], BF16, tag="tq", name="tq")
                                    nc.vector.tensor_add(
                                        tq, pT[:, j - 1, :], pT[:, j, :]
                                    )
                                    nc.tensor.matmul(
                                        denps,
                                        ones_mat,
                                        tq,
                                        start=(m == 0),
                                        stop=(m == nkc // 2 - 1),
                                    )
                    recip_sb = p2b.tile([128, 512], F32, tag="recip", name="recip_sb")
                    nc.vector.reciprocal_approx_fast(recip_sb, denps)
                    nc.vector.tensor_mul(AT[h][:, qsl], pvps, recip_sb)
                for f in pending:
                    f()
                pending = oproj_inject_ops(4 * qg) if qg < NRB - 1 else []

            att_pools.close()
            # ---------------- Phase 3: o_proj (partial over head group) ------
            with tc.tile_pool(name="ps_o", bufs=4, space="PSUM") as ps_o:
                for rc in [r for r in range(NKC) if r not in INJECated]:
                    rsl = slice(rc * 128, (rc + 1) * 128)
                    ops = [
                        ps_o.tile([128, 1024], F32, tag="o", name="ops")
                        for _ in range(2)
                    ]
                    for qc in range(NQC):
                        for oc in range(NRB):
                            nc.tensor.matmul(
                                ops[oc // 2][:, (oc % 2) * 512 : (oc % 2) * 512 + 512],
                                AT[qc][:, rsl],
                                wo_sb[qc][:, oc * 512 : (oc + 1) * 512],
                                start=(qc == 0),
                                stop=(qc == NQC - 1),
                            )
                    for half in range(2):
                        ostage = p2b.tile(
                            [128, 1024], BF16, tag="ost", name="ostage", bufs=4
                        )
                        # alternate engines: ScalarE may still be draining
                        # attention exps when o_proj starts
                        if half == 1:
                            nc.vector.tensor_copy(ostage, ops[half])
                        else:
                            nc.scalar.copy(ostage, ops[half])
                        eng = nc.sync if half == 0 else nc.gpsimd
                        eng.dma_start(
                            out=out[rsl, half * 1024 : half * 1024 + 1024], in_=ostage
                        )

    nc.compile()
    return nc


def _get_nc():
    global _NC
    if _NC is None:
        _NC = _build_nc()
    return _NC


def _make_in_maps(x, wq, bq, wk, bk, wv, bv, wo):
    qli = np.arange(512, dtype=np.int32)[None, :]
    kli = np.arange(128, dtype=np.int32)[:, None]
    mask_np = np.stack(
        [(qli >= kli + 128 * vv) for vv in range(4)]
    ).astype(NPBF16)

    in_maps = []
    for b in range(B):
        xTb = np.ascontiguousarray(np.asarray(x[b], dtype=np.float32).T).astype(NPBF16)
        for g in range(N_KV):
            in_maps.append(
                {
                    "xT": xTb,
                    "wq": np.ascontiguousarray(wq[:, g * 512 : (g + 1) * 512]).astype(
                        NPBF16
                    ),
                    "wk": np.ascontiguousarray(wk[:, g * 128 : (g + 1) * 128]).astype(
                        NPBF16
                    ),
                    "wv": np.ascontiguousarray(wv[:, g * 128 : (g + 1) * 128]).astype(
                        NPBF16
                    ),
                    "wo": np.ascontiguousarray(wo[g * 512 : (g + 1) * 512, :]).astype(
                        NPBF16
                    ),
                    "bq": np.asarray(bq[g * 512 : (g + 1) * 512], dtype=np.float32)
                    .reshape(512, 1)
                    .copy(),
                    "bk": np.asarray(bk[g * 128 : (g + 1) * 128], dtype=np.float32)
                    .reshape(128, 1)
                    .copy(),
                    "bv": np.asarray(bv[g * 128 : (g + 1) * 128], dtype=np.float32)
                    .reshape(128, 1)
                    .copy(),
                    "masks": mask_np,
                }
            )
    return in_maps


def run_device(x, wq, bq, wk, bk, wv, bv, wo, trace=False):
    """Run the SPMD kernel; returns (full_output, BassKernelResults)."""
    nc = _get_nc()
    in_maps = _make_in_maps(x, wq, bq, wk, bk, wv, bv, wo)
    res = run_bass_kernel_spmd(nc, in_maps, core_ids=list(range(8)), trace=trace)
    out = np.zeros((B, S, H), dtype=np.float32)
    for b in range(B):
        for g in range(N_KV):
            out[b] += res.results[b * N_KV + g]["out"].astype(np.float32)
    return out, res


def kernel(x, attention_mask, position_ids, wq, bq, wk, bk, wv, bv, wo):
    del attention_mask, position_ids  # causal mask is built on-chip
    x = np.asarray(x, dtype=np.float32)
    out, _ = run_device(
        x,
        np.asarray(wq, np.float32),
        np.asarray(bq, np.float32),
        np.asarray(wk, np.float32),
        np.asarray(bk, np.float32),
        np.asarray(wv, np.float32),
        np.asarray(bv, np.float32),
        np.asarray(wo, np.float32),
    )
    return out
